# revision 6
# baseline (speedup 1.0000x reference)
"""ClusterNorm2d Trainium2 kernel.

Reference semantics (see problem): per-(cluster, channel) statistics over
(batch members of the cluster) x (spatial), blended 0.2/0.8 with running
stats, then per-sample affine normalization.

Sharding: channel-parallel across the 8 NeuronCores (8 channels each).
Cluster statistics for a channel only ever combine values of that same
channel across the batch, so each core computes its channels' statistics
independently -- no cross-core collective is needed at all.

Per-core layout: the [64, 8, 112, 112] channel shard is viewed as
[512 rows = (b, c), 12544 = H*W] and held SBUF-resident in 4 tiles of
[128, 12544] f32 (~196 KiB/partition), so x is read from HBM exactly once:

  DMA in -> DVE row sums (2-level reduce) + ACT Square accum_out (sum sq)
  -> tiny PE matmul vs host-built one-hot (segment sum over batch)
  -> tiny stats chain (blend, sqrt, reciprocal)  [all label math folded on
     host into per-(cluster,channel) coefficient vectors]
  -> tiny PE matmul gather (per-row scale/offset)
  -> in-place DVE fused affine (x*scale + offset) -> DMA out.
"""

import os
import sys

import numpy as np

for _p in ("/opt/trn_rl_repo",):
    if _p not in sys.path and os.path.isdir(_p):
        sys.path.insert(0, _p)

import concourse.bacc as bacc
import concourse.bass as bass
import concourse.tile as tile
from concourse import mybir
from concourse.bass_utils import run_bass_kernel_spmd

EPS = 1e-05
N_CLUSTERS = 4
B, C, H, W = 64, 64, 112, 112
HW = H * W                      # 12544
N_CORES = 8
CS = C // N_CORES               # 8 channels per core
R = B * CS                      # 512 rows per core
P = 128                         # SBUF partitions
NT = R // P                     # 4 row tiles per core
BT = P // CS                    # 16 batch samples per row tile
GC = N_CLUSTERS * CS            # 32 (cluster, local-channel) pairs
SQ_CHUNK = 896                  # ACT square chunk (fits 2 PSUM banks)
NCH = HW // SQ_CHUNK            # 14 chunks
RED_B = 128                     # inner width of 2-level row-sum reduce
RED_A = HW // RED_B             # 98

_F32 = mybir.dt.float32

_CACHE = {}


def _build_nc(n_iters=1):
    """Build + compile the single-core Bass program (SPMD across 8 cores).

    n_iters > 1 repeats the whole body (used only for benchmarking: the
    in-NEFF loop lets per-iteration HW time be measured as a wall-clock
    delta, cancelling the PJRT/axon dispatch overhead).
    """
    nc = bacc.Bacc("TRN2", target_bir_lowering=False, debug=False)

    x = nc.dram_tensor("x", [R, HW], _F32, kind="ExternalInput")
    oh = nc.dram_tensor("oh", [NT, P, GC], _F32, kind="ExternalInput")
    gs = nc.dram_tensor("gs", [NT, GC, P], _F32, kind="ExternalInput")
    par = nc.dram_tensor("par", [GC, 8], _F32, kind="ExternalInput")
    y = nc.dram_tensor("y", [R, HW], _F32, kind="ExternalOutput")

    AX = mybir.AxisListType.X
    ADD = mybir.AluOpType.add

    with tile.TileContext(nc) as tc:
        with (
            tc.tile_pool(name="consts", bufs=1) as consts,
            tc.tile_pool(name="xpool", bufs=NT) as xpool,
            tc.tile_pool(name="stats", bufs=NT) as stats,
            tc.tile_pool(name="small", bufs=1) as small,
            tc.tile_pool(name="pscr", bufs=2, space="PSUM") as pscr,
            tc.tile_pool(name="pacc", bufs=1, space="PSUM") as pacc,
            tc.tile_pool(name="psc", bufs=2, space="PSUM") as psc,
        ):
            sb_oh = consts.tile([P, NT, GC], _F32)
            nc.sync.dma_start(out=sb_oh, in_=oh.rearrange("t k j -> k t j"))
            sb_gs = consts.tile([GC, NT, P], _F32)
            nc.sync.dma_start(out=sb_gs, in_=gs.rearrange("t j k -> j t k"))
            sb_par = consts.tile([GC, 8], _F32)
            nc.sync.dma_start(out=sb_par, in_=par[:])

            for _ in range(n_iters):
                _emit_iter(nc, tc, x, y, sb_oh, sb_gs, sb_par,
                           xpool, stats, small, pscr, pacc, psc, AX, ADD)

    nc.compile()
    return nc


def _emit_iter(nc, tc, x, y, sb_oh, sb_gs, sb_par,
               xpool, stats, small, pscr, pacc, psc, AX, ADD):
    if True:
        if True:
            xt = []
            for t in range(NT):
                xtile = xpool.tile([P, HW], _F32, tag="x")
                nc.sync.dma_start(out=xtile, in_=x[t * P:(t + 1) * P, :])
                xt.append(xtile)

            # --- pass 1: per-row sum and sum-of-squares, segment-summed on
            # the fly into PSUM via one-hot matmuls ------------------------
            psum_acc = pacc.tile([GC, 2], _F32)
            for t in range(NT):
                ss_t = stats.tile([P, 2], _F32, tag="s_ss")
                part = stats.tile([P, RED_A], _F32, tag="part")
                nc.vector.tensor_reduce(
                    part,
                    xt[t].rearrange("p (a b) -> p a b", b=RED_B),
                    axis=AX,
                    op=ADD,
                )
                nc.vector.tensor_reduce(ss_t[:, 0:1], part, axis=AX, op=ADD)

                sqp = stats.tile([P, NCH], _F32, tag="sqp")
                for ch in range(NCH):
                    scr = pscr.tile([P, SQ_CHUNK], _F32, tag="scr")
                    nc.scalar.activation(
                        out=scr,
                        in_=xt[t][:, ch * SQ_CHUNK:(ch + 1) * SQ_CHUNK],
                        func=mybir.ActivationFunctionType.Square,
                        accum_out=sqp[:, ch:ch + 1],
                    )
                nc.vector.tensor_reduce(ss_t[:, 1:2], sqp, axis=AX, op=ADD)

                nc.tensor.matmul(
                    psum_acc,
                    lhsT=sb_oh[:, t, :],
                    rhs=ss_t,
                    start=(t == 0),
                    stop=(t == NT - 1),
                )

            # --- cluster stats -> per-(g,cl) scale/offset -----------------
            # par columns: 0:c_mean 1:cA 2:cB 3:rv08(+eps) 4:rm08 5:w 6:b
            st = small.tile([GC, 8], _F32)
            so32 = small.tile([GC, 2], _F32)
            mean = st[:, 0:1]
            q2 = st[:, 1:2]
            varb = st[:, 2:3]
            tmp = st[:, 3:4]
            std = st[:, 4:5]
            rstd = st[:, 5:6]
            mu = st[:, 6:7]
            nc.vector.tensor_mul(mean, psum_acc[:, 0:1], sb_par[:, 0:1])
            nc.vector.tensor_mul(q2, mean, mean)
            nc.vector.tensor_mul(varb, psum_acc[:, 1:2], sb_par[:, 1:2])
            nc.vector.tensor_mul(tmp, q2, sb_par[:, 2:3])
            nc.vector.tensor_sub(varb, varb, tmp)
            nc.vector.tensor_add(varb, varb, sb_par[:, 3:4])
            nc.scalar.activation(
                out=std, in_=varb, func=mybir.ActivationFunctionType.Sqrt
            )
            nc.vector.reciprocal(rstd, std)
            nc.vector.tensor_mul(so32[:, 0:1], rstd, sb_par[:, 5:6])
            nc.vector.tensor_scalar(
                out=mu,
                in0=mean,
                scalar1=0.2,
                scalar2=sb_par[:, 4:5],
                op0=mybir.AluOpType.mult,
                op1=ADD,
            )
            nc.vector.tensor_mul(tmp, mu, so32[:, 0:1])
            nc.vector.tensor_sub(so32[:, 1:2], sb_par[:, 6:7], tmp)

            # --- pass 2: scatter scale/offset to rows, fused affine, out --
            for t in range(NT):
                pso = psc.tile([P, 2], _F32, tag="pso")
                nc.tensor.matmul(
                    pso, lhsT=sb_gs[:, t, :], rhs=so32, start=True, stop=True
                )
                so_t = stats.tile([P, 2], _F32, tag="so_t")
                nc.vector.tensor_copy(so_t, pso)
                nc.vector.tensor_scalar(
                    out=xt[t],
                    in0=xt[t],
                    scalar1=so_t[:, 0:1],
                    scalar2=so_t[:, 1:2],
                    op0=mybir.AluOpType.mult,
                    op1=ADD,
                )
                nc.sync.dma_start(out=y[t * P:(t + 1) * P, :], in_=xt[t])


def host_prep(x, running_mean, running_var, weight, bias, labels):
    """Fold all label math into per-core input tensors. Returns in_maps."""
    labels = np.asarray(labels).astype(np.int64)
    x = np.asarray(x, dtype=np.float32)

    cnt = np.bincount(labels, minlength=N_CLUSTERS).astype(np.float64)
    N = cnt * HW
    c_mean = 1.0 / np.maximum(N, 1.0)
    denom = np.maximum(N - 1.0, 1.0)
    cA = 0.2 / denom
    cB = 0.2 * N / denom

    # one-hot segment matrices, identical for every core
    oh = np.zeros((NT, P, GC), dtype=np.float32)
    gs = np.zeros((NT, GC, P), dtype=np.float32)
    for t in range(NT):
        for k in range(P):
            b = t * BT + k // CS
            cl = k % CS
            j = labels[b] * CS + cl
            oh[t, k, j] = 1.0
            gs[t, j, k] = 1.0

    g_idx = np.repeat(np.arange(N_CLUSTERS), CS)          # [GC]
    in_maps = []
    for i in range(N_CORES):
        c_idx = i * CS + np.tile(np.arange(CS), N_CLUSTERS)
        par = np.zeros((GC, 8), dtype=np.float32)
        par[:, 0] = c_mean[g_idx]
        par[:, 1] = cA[g_idx]
        par[:, 2] = cB[g_idx]
        par[:, 3] = 0.8 * np.asarray(running_var, np.float64)[c_idx] + EPS
        par[:, 4] = 0.8 * np.asarray(running_mean, np.float64)[c_idx]
        par[:, 5] = np.asarray(weight, np.float32)[c_idx]
        par[:, 6] = np.asarray(bias, np.float32)[c_idx]
        xs = np.ascontiguousarray(
            x[:, i * CS:(i + 1) * CS].reshape(R, HW)
        )
        in_maps.append({"x": xs, "oh": oh, "gs": gs, "par": par})
    return in_maps


def get_nc(n_iters=1):
    key = ("nc", n_iters)
    if key not in _CACHE:
        _CACHE[key] = _build_nc(n_iters)
    return _CACHE[key]


def kernel(x, running_mean, running_var, weight, bias, labels, **run_kwargs):
    nc = get_nc()
    in_maps = host_prep(x, running_mean, running_var, weight, bias, labels)
    res = run_bass_kernel_spmd(nc, in_maps, list(range(N_CORES)), **run_kwargs)
    outs = [
        res.results[i]["y"].reshape(B, CS, H, W) for i in range(N_CORES)
    ]
    out = np.concatenate(outs, axis=1)
    if run_kwargs:
        kernel.last_results = res
    return out


# revision 7
# speedup vs baseline: 1.2946x; 1.2946x over previous
"""ClusterNorm2d Trainium2 kernel.

Reference semantics (see problem): per-(cluster, channel) statistics over
(batch members of the cluster) x (spatial), blended 0.2/0.8 with running
stats, then per-sample affine normalization.

Sharding: channel-parallel across the 8 NeuronCores (8 channels each).
Cluster statistics for a channel only ever combine values of that same
channel across the batch, so each core computes its channels' statistics
independently -- no cross-core collective is needed at all.

Per-core layout: the [64, 8, 112, 112] channel shard is viewed
channel-major as [512 rows = (c, b), 12544 = H*W] in 4 SBUF-resident
tiles of [128, 12544] f32. Each tile holds 2 *complete* channels
(2 x 64 batch rows), so its cluster statistics are self-contained:
tile t's normalized output DMA can start while tiles t+1.. are still
streaming in, and the DMA engines stay busy back-to-back at the HBM
roofline (x is read from HBM exactly once, y written once).

Per tile:
  DMA in -> DVE row sums + ACT Square accum_out chunks (sum of squares)
  -> tiny PE matmul vs host-built one-hot (segment-sum over batch)
  -> tiny stats chain (blend, sqrt, reciprocal) [all label/count math
     folded on host into per-(channel,cluster) coefficient vectors]
  -> tiny PE matmul gather (per-row scale/offset)
  -> in-place fused DVE affine (x*scale + offset, 2x DVE mode) -> DMA out.
"""

import os
import sys

import numpy as np

for _p in ("/opt/trn_rl_repo",):
    if _p not in sys.path and os.path.isdir(_p):
        sys.path.insert(0, _p)

import concourse.bacc as bacc
import concourse.bass as bass
import concourse.tile as tile
from concourse import mybir
from concourse.bass_utils import run_bass_kernel_spmd

EPS = 1e-05
N_CLUSTERS = 4
B, C, H, W = 64, 64, 112, 112
HW = H * W                      # 12544
N_CORES = 8
CS = C // N_CORES               # 8 channels per core
R = B * CS                      # 512 rows per core
P = 128                         # SBUF partitions
NT = R // P                     # 4 row tiles per core
CT = P // B                     # 2 channels per tile
GC = N_CLUSTERS * CT            # 8 (channel, cluster) pairs per tile
SQ_CHUNK = 896                  # ACT square chunk (fits 2 PSUM banks)
NCH = HW // SQ_CHUNK            # 14 chunks
RED_B = 128                     # inner width of 2-level row-sum reduce
RED_A = HW // RED_B             # 98

_F32 = mybir.dt.float32

_CACHE = {}


def _build_nc(n_iters=1):
    """Build + compile the single-core Bass program (SPMD across 8 cores).

    n_iters > 1 repeats the whole body (used only for benchmarking: the
    in-NEFF loop lets per-iteration HW time be measured as a wall-clock
    delta, cancelling the PJRT/axon dispatch overhead).
    """
    nc = bacc.Bacc("TRN2", target_bir_lowering=False, debug=False)

    x = nc.dram_tensor("x", [R, HW], _F32, kind="ExternalInput")
    oh = nc.dram_tensor("oh", [NT, P, GC], _F32, kind="ExternalInput")
    gs = nc.dram_tensor("gs", [NT, GC, P], _F32, kind="ExternalInput")
    par = nc.dram_tensor("par", [NT * GC, 8], _F32, kind="ExternalInput")
    y = nc.dram_tensor("y", [R, HW], _F32, kind="ExternalOutput")

    with tile.TileContext(nc) as tc:
        with (
            tc.tile_pool(name="consts", bufs=1) as consts,
            tc.tile_pool(name="xpool", bufs=NT) as xpool,
            tc.tile_pool(name="stats", bufs=2 * NT) as stats,
            tc.tile_pool(name="pscr", bufs=2, space="PSUM") as pscr,
            tc.tile_pool(name="pacc", bufs=2, space="PSUM") as pacc,
            tc.tile_pool(name="psc", bufs=2, space="PSUM") as psc,
        ):
            sb_oh = consts.tile([P, NT, GC], _F32)
            nc.sync.dma_start(out=sb_oh, in_=oh.rearrange("t k j -> k t j"))
            sb_gs = consts.tile([GC, NT, P], _F32)
            nc.sync.dma_start(out=sb_gs, in_=gs.rearrange("t j k -> j t k"))
            sb_par = consts.tile([GC, NT, 8], _F32)
            nc.sync.dma_start(
                out=sb_par, in_=par.rearrange("(t j) c -> j t c", j=GC)
            )
            pools = (xpool, stats, pscr, pacc, psc)
            for _ in range(n_iters):
                _emit_iter(nc, x, y, sb_oh, sb_gs, sb_par, pools)

    nc.compile()
    return nc


def _emit_iter(nc, x, y, sb_oh, sb_gs, sb_par, pools):
    xpool, stats, pscr, pacc, psc = pools
    AX = mybir.AxisListType.X
    ADD = mybir.AluOpType.add
    MUL = mybir.AluOpType.mult

    xt = []
    for t in range(NT):
        xtile = xpool.tile([P, HW], _F32, tag="x")
        nc.sync.dma_start(out=xtile, in_=x[t * P:(t + 1) * P, :])
        xt.append(xtile)

    for t in range(NT):
        # --- per-row sum and sum-of-squares --------------------------------
        ss_t = stats.tile([P, 2], _F32, tag="s_ss")
        part = stats.tile([P, RED_A], _F32, tag="part")
        nc.vector.tensor_reduce(
            part,
            xt[t].rearrange("p (a b) -> p a b", b=RED_B),
            axis=AX,
            op=ADD,
        )
        nc.vector.tensor_reduce(ss_t[:, 0:1], part, axis=AX, op=ADD)

        sqp = stats.tile([P, NCH], _F32, tag="sqp")
        for ch in range(NCH):
            scr = pscr.tile([P, SQ_CHUNK], _F32, tag="scr")
            nc.scalar.activation(
                out=scr,
                in_=xt[t][:, ch * SQ_CHUNK:(ch + 1) * SQ_CHUNK],
                func=mybir.ActivationFunctionType.Square,
                accum_out=sqp[:, ch:ch + 1],
            )
        nc.vector.tensor_reduce(ss_t[:, 1:2], sqp, axis=AX, op=ADD)

        # --- segment-sum over the 64 batch rows of each channel ------------
        psum_acc = pacc.tile([GC, 2], _F32, tag="acc")
        nc.tensor.matmul(
            psum_acc, lhsT=sb_oh[:, t, :], rhs=ss_t, start=True, stop=True
        )

        # --- cluster stats -> per-(channel,cluster) scale/offset -----------
        # par columns: 0:c_mean 1:cA 2:cB 3:rv08(+eps) 4:rm08 5:w 6:b
        pt = sb_par[:, t, :]
        st = stats.tile([GC, 8], _F32, tag="st")
        so8 = stats.tile([GC, 2], _F32, tag="so8")
        mean = st[:, 0:1]
        q2 = st[:, 1:2]
        varb = st[:, 2:3]
        tmp = st[:, 3:4]
        std = st[:, 4:5]
        rstd = st[:, 5:6]
        mu = st[:, 6:7]
        nc.vector.tensor_mul(mean, psum_acc[:, 0:1], pt[:, 0:1])
        nc.vector.tensor_mul(q2, mean, mean)
        nc.vector.tensor_mul(varb, psum_acc[:, 1:2], pt[:, 1:2])
        nc.vector.tensor_mul(tmp, q2, pt[:, 2:3])
        nc.vector.tensor_sub(varb, varb, tmp)
        nc.vector.tensor_add(varb, varb, pt[:, 3:4])
        nc.scalar.activation(
            out=std, in_=varb, func=mybir.ActivationFunctionType.Sqrt
        )
        nc.vector.reciprocal(rstd, std)
        nc.vector.tensor_mul(so8[:, 0:1], rstd, pt[:, 5:6])
        nc.vector.tensor_scalar(
            out=mu, in0=mean, scalar1=0.2, scalar2=pt[:, 4:5],
            op0=MUL, op1=ADD,
        )
        nc.vector.tensor_mul(tmp, mu, so8[:, 0:1])
        nc.vector.tensor_sub(so8[:, 1:2], pt[:, 6:7], tmp)

        # --- scatter scale/offset to rows, fused affine, store -------------
        pso = psc.tile([P, 2], _F32, tag="pso")
        nc.tensor.matmul(
            pso, lhsT=sb_gs[:, t, :], rhs=so8, start=True, stop=True
        )
        so_t = stats.tile([P, 2], _F32, tag="so_t")
        nc.vector.tensor_copy(so_t, pso)
        nc.vector.tensor_scalar(
            out=xt[t],
            in0=xt[t],
            scalar1=so_t[:, 0:1],
            scalar2=so_t[:, 1:2],
            op0=MUL,
            op1=ADD,
        )
        nc.sync.dma_start(out=y[t * P:(t + 1) * P, :], in_=xt[t])


def host_prep(x, running_mean, running_var, weight, bias, labels):
    """Fold all label math into per-core input tensors. Returns in_maps."""
    labels = np.asarray(labels).astype(np.int64)
    x = np.asarray(x, dtype=np.float32)

    cnt = np.bincount(labels, minlength=N_CLUSTERS).astype(np.float64)
    N = cnt * HW
    c_mean = 1.0 / np.maximum(N, 1.0)
    denom = np.maximum(N - 1.0, 1.0)
    cA = 0.2 / denom
    cB = 0.2 * N / denom

    # Row layout per core: r = cl*B + b (channel-major).  Tile t holds
    # channels {2t, 2t+1}; within the tile, row k -> (cl_local = k//B,
    # b = k%B); stats slot j = cl_local*N_CLUSTERS + g.
    oh = np.zeros((NT, P, GC), dtype=np.float32)
    gs = np.zeros((NT, GC, P), dtype=np.float32)
    k = np.arange(P)
    for t in range(NT):
        j = (k // B) * N_CLUSTERS + labels[k % B]
        oh[t, k, j] = 1.0
        gs[t, j, k] = 1.0

    # par rows: (t, j) -> channel c = core*CS + 2t + j//N_CLUSTERS,
    # cluster g = j % N_CLUSTERS
    jj = np.arange(GC)
    g_of_j = jj % N_CLUSTERS
    rm = np.asarray(running_mean, np.float64)
    rv = np.asarray(running_var, np.float64)
    wt = np.asarray(weight, np.float32)
    bs = np.asarray(bias, np.float32)

    in_maps = []
    for i in range(N_CORES):
        par = np.zeros((NT * GC, 8), dtype=np.float32)
        for t in range(NT):
            c_of_j = i * CS + 2 * t + jj // N_CLUSTERS
            rows = slice(t * GC, (t + 1) * GC)
            par[rows, 0] = c_mean[g_of_j]
            par[rows, 1] = cA[g_of_j]
            par[rows, 2] = cB[g_of_j]
            par[rows, 3] = 0.8 * rv[c_of_j] + EPS
            par[rows, 4] = 0.8 * rm[c_of_j]
            par[rows, 5] = wt[c_of_j]
            par[rows, 6] = bs[c_of_j]
        xs = np.ascontiguousarray(
            x[:, i * CS:(i + 1) * CS].transpose(1, 0, 2, 3).reshape(R, HW)
        )
        in_maps.append({"x": xs, "oh": oh, "gs": gs, "par": par})
    return in_maps


def get_nc(n_iters=1):
    key = ("nc", n_iters)
    if key not in _CACHE:
        _CACHE[key] = _build_nc(n_iters)
    return _CACHE[key]


def assemble_out(per_core_y):
    """[N_CORES] x [R, HW] channel-major shards -> [B, C, H, W]."""
    full = np.concatenate(
        [yc.reshape(CS, B, H, W) for yc in per_core_y], axis=0
    )  # [C, B, H, W]
    return np.ascontiguousarray(full.transpose(1, 0, 2, 3))


def kernel(x, running_mean, running_var, weight, bias, labels, **run_kwargs):
    nc = get_nc()
    in_maps = host_prep(x, running_mean, running_var, weight, bias, labels)
    res = run_bass_kernel_spmd(nc, in_maps, list(range(N_CORES)), **run_kwargs)
    out = assemble_out([res.results[i]["y"] for i in range(N_CORES)])
    if run_kwargs:
        kernel.last_results = res
    return out
